# revision 1
# baseline (speedup 1.0000x reference)
"""Bass/Trainium2 kernel for nn_EquivariantPosUpdate — 8-core edge-parallel.

Structure (per core, 1024 edges in 8 tiles of 128):
  setup: load/fold weights, build replicated constant rows, identity, iota
  phase A: node projections -> DRAM proj_src/proj_dst; time-mod table -> DRAM
  phase B: per edge tile: RBF -> two radial MLPs -> per-edge TP-weight chunks
           (PE matmul) consumed by mul+reduce (DVE) -> irrep epilogues ->
           node-fusion linear -> edge-fusion TP (ss+v0 only) -> adaLN ->
           scalar head -> force -> one-hot scatter matmuls into PSUM
  final: evac accumulator -> out [2048, 3] (host sums the 8 partials)
"""
import sys
sys.path.insert(0, '/opt/trn_rl_repo')
import numpy as np
from contextlib import ExitStack

import concourse.bass as bass
import concourse.bacc as bacc
import concourse.mybir as mybir
import concourse.tile as tile
from concourse.bass import AP, IndirectOffsetOnAxis
from concourse.masks import make_identity

F32 = mybir.dt.float32
I32 = mybir.dt.int32
AX = mybir.AxisListType
OP = mybir.AluOpType
ACTF = mybir.ActivationFunctionType

N, E, G, NB = 2048, 8192, 64, 128
NC_CORES = 8
EC = E // NC_CORES          # 1024
P = 128
TILES = EC // P             # 8
M0, M1 = 64, 32
S_TP = 96
CUTOFF = 5.0
DEBUG = False
NCHUNK = N // P             # 16

# rows-packed constant layout (all replicated to 128 partitions on device)
ROWS = {}
_off = 0
for _n, _w in [('nf_g1', 64), ('nf_b1', 64), ('nf_g2', 64), ('nf_b2', 64),
               ('ef_g1', 64), ('ef_b1', 64), ('ef_g2', 64), ('ef_b2', 64),
               ('src_bs', 64), ('dst_bs', 64), ('nt_bs', 64), ('et_bs', 64),
               ('nf_bias', 96), ('ef_bias', 96), ('sp_b1', 32), ('spW2r', 32),
               ('sp_b2', 1), ('eps', 1), ('normbt', 192)]:
    ROWS[_n] = (_off, _w)
    _off += _w
RWID = _off


def rows_slice(rep, name):
    off, w = ROWS[name]
    return rep[:, off:off + w]


def ap3(t, dims, offset=0):
    """Free-dim AP with explicit [step, count] dims on an SBUF/PSUM tile."""
    base = t[:, :] if not isinstance(t, AP) else t
    ap = AP(base.tensor, base.offset + offset, [base.ap[0]] + [list(d) for d in dims])
    return ap


def build_nc():
    nc = bacc.Bacc("TRN2", target_bir_lowering=False, debug=False,
                   num_devices=NC_CORES)
    T = {}

    def din(name, shape, dtype=F32):
        T[name] = nc.dram_tensor(name, shape, dtype, kind="ExternalInput")
        return T[name]

    # --- inputs ---
    din('hn_T', [320, N]); din('he_T', [160, EC])
    din('dist', [EC, 1]); din('rvec', [EC, 3]); din('srcf', [EC, 1])
    din('srci', [EC, 1], I32); din('dsti', [EC, 1], I32); din('gidi', [EC, 1], I32)
    din('t_T', [128, G]); din('normWt', [128, 2 * S_TP])
    din('rows', [1, RWID])
    din('rbf_mean_r', [1, NB]); din('rbf_std_r', [1, NB]); din('rbf_std_c', [NB, 1])
    din('rbf_w', [1, 1]); din('rbf_b', [1, 1])
    for p in ('nf', 'ef'):
        din(p + '_W1', [NB, 64]); din(p + '_W2', [64, 64])
    din('W3nf', [64, 10240]); din('W3ef', [64, 5120])
    din('src_Ws', [128, 64]); din('dst_Ws', [128, 64])
    din('src_Wv', [64, 32]); din('dst_Wv', [64, 32])
    din('nt_Ws', [S_TP, 64]); din('nt_Wv', [128, 32])
    din('et_Ws', [64, 64]); din('et_Wv', [32, 32])
    din('sp_W1', [S_TP, 32])
    out = nc.dram_tensor('out', [N, 3], F32, kind="ExternalOutput")
    T['out'] = out
    # DRAM scratch
    T['proj_src'] = nc.dram_tensor('proj_src', [N, 160], F32)
    T['proj_dst'] = nc.dram_tensor('proj_dst', [N, 160], F32)
    T['mod_d'] = nc.dram_tensor('mod_d', [G, 2 * S_TP], F32)
    if DEBUG:
        for nm, sh in [('dbg_force', [EC, 3]), ('dbg_fs', [EC, S_TP]),
                       ('dbg_as', [EC, S_TP]), ('dbg_gsrc', [EC, 160]),
                       ('dbg_h2', [64, EC]), ('dbg_fv', [EC, 384]),
                       ('dbg_ns', [EC, 64]), ('dbg_nv', [EC, 96]),
                       ('dbg_sn', [EC, S_TP]), ('dbg_eset', [EC, 64]),
                       ('dbg_evet', [EC, 96])]:
            T[nm] = nc.dram_tensor(nm, sh, F32, kind="ExternalOutput")

    with tile.TileContext(nc) as tc:
        with ExitStack() as ctx:
            _build(ctx, tc, nc, T)
    nc.compile()
    return nc


def _build(ctx, tc, nc, T):
    consts = ctx.enter_context(tc.tile_pool(name="consts", bufs=1))
    setup = ctx.enter_context(tc.tile_pool(name="setup", bufs=2))
    sb = ctx.enter_context(tc.tile_pool(name="sb", bufs=3))
    sbq = ctx.enter_context(tc.tile_pool(name="sbq", bufs=3))
    sbg = ctx.enter_context(tc.tile_pool(name="sbg", bufs=2))
    ps = ctx.enter_context(tc.tile_pool(name="ps", bufs=4, space="PSUM"))
    psw = ctx.enter_context(tc.tile_pool(name="psw", bufs=3, space="PSUM"))
    psa = ctx.enter_context(tc.tile_pool(name="psa", bufs=1, space="PSUM"))
    dma = nc.sync.dma_start

    def load(name, shape=None, pool=consts, dt=F32):
        t = pool.tile(shape or T[name].shape, dt, tag="ld_" + name,
                      name="ld_" + name)
        dma(t[:], T[name][:])
        return t

    # ---------------- setup ----------------
    ident = consts.tile([P, P], F32)
    make_identity(nc, ident[:])
    iota_i = consts.tile([P, P], I32)
    nc.gpsimd.iota(iota_i[:], pattern=[[1, P]], base=0, channel_multiplier=0)
    iota_f = consts.tile([P, P], F32)
    nc.vector.tensor_copy(iota_f[:], iota_i[:])

    rows1 = consts.tile([1, RWID], F32)
    dma(rows1[:], T['rows'][:])
    # normbt scale-slot gets +1 (adaLN 1+scale fold)
    o_nbt = ROWS['normbt'][0]
    nc.vector.tensor_scalar_add(rows1[:, o_nbt + S_TP:o_nbt + 2 * S_TP],
                                rows1[:, o_nbt + S_TP:o_nbt + 2 * S_TP], 1.0)
    rep = consts.tile([P, RWID], F32)
    nc.gpsimd.partition_broadcast(rep[:], rows1[:])

    # RBF constants
    stdr = load('rbf_std_r', pool=setup); meanr = load('rbf_mean_r', pool=setup)
    rw = load('rbf_w', pool=setup); rb = load('rbf_b', pool=setup)
    invstd = setup.tile([1, NB], F32)
    nc.vector.reciprocal(invstd[:], stdr[:])
    arow = setup.tile([1, NB], F32)
    nc.vector.tensor_scalar(arow[:], invstd[:], rw[:, :1], 1.0 / CUTOFF,
                            op0=OP.mult, op1=OP.mult)
    minv = setup.tile([1, NB], F32)
    nc.vector.tensor_mul(minv[:], meanr[:], invstd[:])
    brow = setup.tile([1, NB], F32)
    nc.vector.scalar_tensor_tensor(brow[:], invstd[:], rb[:, :1], minv[:],
                                   op0=OP.mult, op1=OP.subtract)
    A_rep = consts.tile([P, NB], F32); B_rep = consts.tile([P, NB], F32)
    nc.gpsimd.partition_broadcast(A_rep[:], arow[:])
    nc.gpsimd.partition_broadcast(B_rep[:], brow[:])

    stdc = load('rbf_std_c', pool=setup)
    ccol = setup.tile([NB, 1], F32)
    nc.vector.reciprocal(ccol[:], stdc[:])
    nc.vector.tensor_scalar_mul(ccol[:], ccol[:], 1.0 / np.sqrt(2 * np.pi))

    W1p = consts.tile([NB, 128], F32)
    dma(W1p[:, 0:64], T['nf_W1'][:]); dma(W1p[:, 64:128], T['ef_W1'][:])
    nc.vector.tensor_scalar(W1p[:], W1p[:], ccol[:, :1], None, op0=OP.mult)
    W2nf = load('nf_W2'); W2ef = load('ef_W2')
    W3nf = load('W3nf'); W3ef = load('W3ef')

    Wsd = consts.tile([128, 128], F32)
    dma(Wsd[:, 0:64], T['src_Ws'][:]); dma(Wsd[:, 64:128], T['dst_Ws'][:])
    nc.vector.tensor_scalar_mul(Wsd[:], Wsd[:], 128.0 ** -0.5)
    Wvsd = consts.tile([64, 64], F32)
    dma(Wvsd[:, 0:32], T['src_Wv'][:]); dma(Wvsd[:, 32:64], T['dst_Wv'][:])
    nc.vector.tensor_scalar_mul(Wvsd[:], Wvsd[:], 64.0 ** -0.5)
    ntWs = load('nt_Ws'); nc.vector.tensor_scalar_mul(ntWs[:], ntWs[:], 96.0 ** -0.5)
    ntWv = load('nt_Wv'); nc.vector.tensor_scalar_mul(ntWv[:], ntWv[:], 128.0 ** -0.5)
    etWs = load('et_Ws'); nc.vector.tensor_scalar_mul(etWs[:], etWs[:], 64.0 ** -0.5)
    etWv = load('et_Wv'); nc.vector.tensor_scalar_mul(etWv[:], etWv[:], 32.0 ** -0.5)
    spW1 = load('sp_W1'); nc.vector.tensor_scalar_mul(spW1[:], spW1[:], 96.0 ** -0.5)
    normWt = load('normWt')
    tT = load('t_T')

    def evac_add(dst, src_ps, bias_ap):
        nc.vector.tensor_tensor(dst, src_ps, bias_ap, op=OP.add)

    _silu_n = [0]

    def silu(dst, src_ap, width, pool):
        _silu_n[0] += 1
        sg = pool.tile([P, width], F32, tag="silu_sg", name=f"sg_{_silu_n[0]}")
        nc.scalar.activation(sg[:], src_ap, ACTF.Sigmoid)
        nc.vector.tensor_mul(dst, sg[:], src_ap)

    # ---------------- phase A: node projections ----------------
    for c in range(NCHUNK):
        hsT = setup.tile([128, P], F32, tag="hsT")
        dma(hsT[:], T['hn_T'][0:128, c * P:(c + 1) * P])
        pp = ps.tile([P, 128], F32, tag="ps_small")
        nc.tensor.matmul(pp[:], hsT[:], Wsd[:], start=True, stop=True)
        ssb = setup.tile([P, 128], F32, tag="projs")
        evac_add(ssb[:], pp[:], rep[:, ROWS['src_bs'][0]:ROWS['src_bs'][0] + 128])
        dma(T['proj_src'][c * P:(c + 1) * P, 0:64], ssb[:, 0:64])
        dma(T['proj_dst'][c * P:(c + 1) * P, 0:64], ssb[:, 64:128])
        for x in range(3):
            hvT = setup.tile([64, P], F32, tag="hvT")
            dma(hvT[:], T['hn_T'][128 + x:320:3, c * P:(c + 1) * P])
            pv = ps.tile([P, 64], F32, tag="ps_small")
            nc.tensor.matmul(pv[:], hvT[:], Wvsd[:], start=True, stop=True)
            vsb = setup.tile([P, 64], F32, tag="projv")
            nc.scalar.copy(vsb[:], pv[:])
            dma(T['proj_src'][c * P:(c + 1) * P, 64 + 32 * x:96 + 32 * x], vsb[:, 0:32])
            dma(T['proj_dst'][c * P:(c + 1) * P, 64 + 32 * x:96 + 32 * x], vsb[:, 32:64])

    # mod table
    pm = ps.tile([G, 2 * S_TP], F32, tag="ps_small")
    nc.tensor.matmul(pm[:], tT[:], normWt[:], start=True, stop=True)
    msb = setup.tile([G, 2 * S_TP], F32)
    evac_add(msb[:], pm[:], rep[0:G, ROWS['normbt'][0]:ROWS['normbt'][0] + 2 * S_TP])
    dma(T['mod_d'][:], msb[:])

    # ---------------- phase B: edge tiles ----------------
    acc_sb = consts.tile([P, NCHUNK * 3], F32)
    nc.vector.memset(acc_sb[:], 0.0)

    for ti in range(TILES):
        e0 = ti * P
        d_col = sb.tile([P, 1], F32, tag="dcol")
        dma(d_col[:], T['dist'][e0:e0 + P, :])
        rv = sb.tile([P, 3], F32, tag="rv")
        dma(rv[:], T['rvec'][e0:e0 + P, :])
        srcf = sb.tile([P, 1], F32, tag="srcf")
        dma(srcf[:], T['srcf'][e0:e0 + P, :])
        si = sb.tile([P, 1], I32, tag="si")
        dma(si[:], T['srci'][e0:e0 + P, :])
        di = sb.tile([P, 1], I32, tag="di")
        dma(di[:], T['dsti'][e0:e0 + P, :])
        gi = sb.tile([P, 1], I32, tag="gi")
        dma(gi[:], T['gidi'][e0:e0 + P, :])

        g_src = sbg.tile([P, 160], F32, tag="gsrc")
        nc.gpsimd.indirect_dma_start(
            out=g_src[:], out_offset=None, in_=T['proj_src'][:],
            in_offset=IndirectOffsetOnAxis(ap=si[:, :1], axis=0))
        g_dst = sbg.tile([P, 160], F32, tag="gdst")
        nc.gpsimd.indirect_dma_start(
            out=g_dst[:], out_offset=None, in_=T['proj_dst'][:],
            in_offset=IndirectOffsetOnAxis(ap=di[:, :1], axis=0))
        g_mod = sbg.tile([P, 2 * S_TP], F32, tag="gmod")
        nc.gpsimd.indirect_dma_start(
            out=g_mod[:], out_offset=None, in_=T['mod_d'][:],
            in_offset=IndirectOffsetOnAxis(ap=gi[:, :1], axis=0))

        heT = sb.tile([64, P], F32, tag="heT")
        dma(heT[:], T['he_T'][0:64, e0:e0 + P])
        hevT = [sb.tile([32, P], F32, tag=f"hevT{x}", name=f"hevT{x}_{ti}")
                for x in range(3)]
        for x in range(3):
            dma(hevT[x][:], T['he_T'][64 + x:160:3, e0:e0 + P])

        # --- RBF ---
        z = sb.tile([P, NB], F32, tag="z")
        nc.vector.scalar_tensor_tensor(z[:], A_rep[:], d_col[:, :1], B_rep[:],
                                       op0=OP.mult, op1=OP.add)
        zsq = sb.tile([P, NB], F32, tag="zsq")
        nc.scalar.square(zsq[:], z[:])
        es_rbf = sb.tile([P, NB], F32, tag="esrbf")
        nc.scalar.activation(es_rbf[:], zsq[:], ACTF.Exp, scale=-0.5)
        esT_p = ps.tile([NB, P], F32, tag="ps_small")
        nc.tensor.transpose(esT_p[:], es_rbf[:], ident[:])
        esT = sb.tile([NB, P], F32, tag="esT")
        nc.scalar.copy(esT[:], esT_p[:])

        # --- radial MLPs (nf | ef share x1 matmul) ---
        x1 = ps.tile([P, 128], F32, tag="ps_small")
        nc.tensor.matmul(x1[:], esT[:], W1p[:], start=True, stop=True)

        def layer_norm(src_ap, gname, bname, dest, width):
            mu = sb.tile([P, 1], F32, tag="lnmu")
            nc.vector.tensor_reduce(mu[:], src_ap, axis=AX.X, op=OP.add)
            nc.vector.tensor_scalar_mul(mu[:], mu[:], 1.0 / width)
            cen = sb.tile([P, width], F32, tag="lncen")
            nc.vector.tensor_scalar(cen[:], src_ap, mu[:, :1], None, op0=OP.subtract)
            sqv = sb.tile([P, width], F32, tag="lnsq")
            var = sb.tile([P, 1], F32, tag="lnvar")
            nc.scalar.activation(sqv[:], cen[:], ACTF.Square, accum_out=var[:])
            std = sb.tile([P, 1], F32, tag="lnstd")
            nc.scalar.activation(std[:], var[:], ACTF.Sqrt, scale=1.0 / width,
                                 bias=rep[:, ROWS['eps'][0]:ROWS['eps'][0] + 1])
            rstd = sb.tile([P, 1], F32, tag="lnrstd")
            nc.vector.reciprocal(rstd[:], std[:])
            nc.vector.scalar_tensor_tensor(dest, cen[:], rstd[:, :1],
                                           rows_slice(rep, gname),
                                           op0=OP.mult, op1=OP.mult)
            nc.vector.tensor_tensor(dest, dest, rows_slice(rep, bname), op=OP.add)

        h2T = {}
        for ri, p in enumerate(('nf', 'ef')):
            hln = sb.tile([P, 64], F32, tag=f"hln{p}")
            layer_norm(x1[:, 64 * ri:64 * ri + 64], p + '_g1', p + '_b1', hln[:], 64)
            h1 = sb.tile([P, 64], F32, tag=f"h1{p}")
            silu(h1[:], hln[:], 64, sb)
            h1T_p = ps.tile([64, P], F32, tag="ps_small")
            nc.tensor.transpose(h1T_p[:], h1[:], ident[:])
            h1T = sb.tile([64, P], F32, tag=f"h1T{p}")
            nc.scalar.copy(h1T[:], h1T_p[:])
            x2 = ps.tile([P, 64], F32, tag="ps_small")
            nc.tensor.matmul(x2[:], h1T[:], (W2nf if p == 'nf' else W2ef)[:],
                             start=True, stop=True)
            h2ln = sb.tile([P, 64], F32, tag=f"h2ln{p}")
            layer_norm(x2[:, :], p + '_g2', p + '_b2', h2ln[:], 64)
            h2 = sb.tile([P, 64], F32, tag=f"h2{p}")
            silu(h2[:], h2ln[:], 64, sb)
            h2T_p = ps.tile([64, P], F32, tag="ps_small")
            nc.tensor.transpose(h2T_p[:], h2[:], ident[:])
            h2T[p] = sb.tile([64, P], F32, tag=f"h2T{p}", name=f"h2T{p}_{ti}")
            nc.scalar.copy(h2T[p][:], h2T_p[:])

        # --- edge transform (es/ev from h_edge) ---
        pe_s = ps.tile([P, 64], F32, tag="ps_small")
        nc.tensor.matmul(pe_s[:], heT[:], etWs[:], start=True, stop=True)
        es_et = sb.tile([P, 64], F32, tag="eset")
        evac_add(es_et[:], pe_s[:], rows_slice(rep, 'et_bs'))
        pe_v = ps.tile([P, 96], F32, tag="ps_small")
        for x in range(3):
            nc.tensor.matmul(pe_v[:, 32 * x:32 * x + 32], hevT[x][:], etWv[:],
                             start=True, stop=True, skip_group_check=True)
        ev_et = sb.tile([P, 96], F32, tag="evet")
        nc.scalar.copy(ev_et[:], pe_v[:])

        s1 = g_src[:, 0:64]; v1 = g_src[:, 64:160]
        s2 = g_dst[:, 0:64]; v2 = g_dst[:, 64:160]

        # --- dtp helper: consume one radial's W3 stream ---
        def dtp(h2T_sb, W3, s_in, v_in, full):
            """Returns dict of bilinear buffers."""
            fl = 'f' if full else 'h'
            r = {}
            r['bil_ss'] = sbq.tile([P, 64], F32, tag="bilss", name=f"bilss{fl}_{ti}")
            if full:
                r['bsv'] = sbq.tile([P, 192], F32, tag="bsv", name=f"bsv_{ti}")
                r['bvs'] = sbq.tile([P, 32], F32, tag="bvs", name=f"bvs_{ti}")
                r['cbuf'] = sbq.tile([P, 96], F32, tag="cbuf", name=f"cbuf_{ti}")
            r['bv0'] = sbq.tile([P, 96], F32, tag="bv0", name=f"bv0{fl}_{ti}")
            nchunks = 20 if full else 10
            for c in range(nchunks):
                pw = psw.tile([P, 512], F32)
                nc.tensor.matmul(pw[:], h2T_sb[:], W3[:, 512 * c:512 * c + 512],
                                 start=True, stop=True)
                if full:
                    kind = ('ss' if c < 8 else 'sv' if c < 12 else
                            'vs' if c < 16 else 'v0' if c < 18 else 'v1')
                    ci = {'ss': c, 'sv': c - 8, 'vs': c - 12,
                          'v0': c - 16, 'v1': c - 18}[kind]
                else:
                    kind = 'ss' if c < 8 else 'v0'
                    ci = c if c < 8 else c - 8
                if kind in ('ss', 'vs'):
                    # chunk = 8 u x 64 v ; mul by s_in bcast over u, reduce v
                    q = sbq.tile([P, 512], F32, tag="qs")
                    nc.vector.tensor_tensor(
                        ap3(q, [[64, 8], [1, 64]]),
                        ap3(pw, [[64, 8], [1, 64]]),
                        ap3(s_in, [[0, 8], [1, 64]]), op=OP.mult)
                    dst = r['bil_ss'] if kind == 'ss' else r['bvs']
                    nc.vector.tensor_reduce(
                        dst[:, 8 * ci:8 * ci + 8],
                        ap3(q, [[64, 8], [1, 64]]), axis=AX.X, op=OP.add)
                else:
                    # chunk = 16 u x 32 v ; q [e,(16u,3x,32v)], reduce v
                    q = sbq.tile([P, 1536], F32, tag="qv")
                    nc.vector.tensor_tensor(
                        ap3(q, [[96, 16], [32, 3], [1, 32]]),
                        ap3(pw, [[32, 16], [0, 3], [1, 32]]),
                        ap3(v_in, [[0, 16], [32, 3], [1, 32]]), op=OP.mult)
                    dst = r['bsv'] if kind == 'sv' else (
                        r['bv0'] if kind == 'v0' else r['cbuf'])
                    nc.vector.tensor_reduce(
                        ap3(dst, [[3, 16], [1, 3]], offset=48 * ci),
                        ap3(q, [[96, 16], [32, 3], [1, 32]]), axis=AX.X, op=OP.add)
            return r

        # ---- dtp1: (s1,v1) x (s2,v2), weights from h2nf ----
        b1r = dtp(h2T['nf'], W3nf, s2, v2, full=True)
        fs = sbq.tile([P, 96], F32, tag="fs")
        fv = sbq.tile([P, 384], F32, tag="fv")
        # out_ss = s1 * bil_ss / 8
        nc.vector.scalar_tensor_tensor(fs[:, 0:64], b1r['bil_ss'][:], 0.125,
                                       s1, op0=OP.mult, op1=OP.mult)
        # out_v0 = sum_x v1*(bv0)/sqrt(96)
        t96 = sbq.tile([P, 96], F32, tag="t96")
        nc.vector.scalar_tensor_tensor(
            ap3(t96, [[3, 32], [1, 3]]),
            ap3(v1, [[1, 32], [32, 3]]), 96.0 ** -0.5,
            ap3(b1r['bv0'], [[3, 32], [1, 3]]), op0=OP.mult, op1=OP.mult)
        nc.vector.tensor_reduce(fs[:, 64:96], ap3(t96, [[3, 32], [1, 3]]),
                                axis=AX.X, op=OP.add)
        nc.vector.tensor_tensor(fs[:], fs[:], rows_slice(rep, 'nf_bias'), op=OP.add)
        # fv sv region: s1 * bsv / sqrt(32)
        nc.vector.scalar_tensor_tensor(
            ap3(fv, [[128, 3], [1, 64]]),
            ap3(b1r['bsv'], [[1, 3], [3, 64]]), 32.0 ** -0.5,
            ap3(s1, [[0, 3], [1, 64]]), op0=OP.mult, op1=OP.mult)
        # fv vs region: v1 * bvs / 8
        nc.vector.scalar_tensor_tensor(
            ap3(fv, [[128, 3], [1, 32]], offset=64),
            ap3(v1, [[32, 3], [1, 32]]), 0.125,
            ap3(b1r['bvs'], [[0, 3], [1, 32]]), op0=OP.mult, op1=OP.mult)
        # fv v1-term region: cross(v1, c)/8
        for x in range(3):
            y, zz = (x + 1) % 3, (x + 2) % 3
            ta = sbq.tile([P, 32], F32, tag="crossa")
            nc.vector.scalar_tensor_tensor(
                ta[:], v1[:, 32 * y:32 * y + 32], 0.125,
                ap3(b1r['cbuf'], [[3, 32]], offset=zz), op0=OP.mult, op1=OP.mult)
            tb = sbq.tile([P, 32], F32, tag="crossb")
            nc.vector.scalar_tensor_tensor(
                tb[:], v1[:, 32 * zz:32 * zz + 32], 0.125,
                ap3(b1r['cbuf'], [[3, 32]], offset=y), op0=OP.mult, op1=OP.mult)
            nc.vector.tensor_sub(fv[:, 128 * x + 96:128 * x + 128], ta[:], tb[:])

        # ---- node-fusion linear ----
        fsT_p = ps.tile([96, P], F32, tag="ps_small")
        nc.tensor.transpose(fsT_p[:], fs[:], ident[:])
        fsT = sbq.tile([96, P], F32, tag="fsT")
        nc.scalar.copy(fsT[:], fsT_p[:])
        ns_p = ps.tile([P, 64], F32, tag="ps_small")
        nc.tensor.matmul(ns_p[:], fsT[:], ntWs[:], start=True, stop=True)
        ns = sbq.tile([P, 64], F32, tag="ns")
        evac_add(ns[:], ns_p[:], rows_slice(rep, 'nt_bs'))
        nv = sbq.tile([P, 96], F32, tag="nv")
        for x in range(3):
            fvT_p = ps.tile([128, P], F32, tag="ps_small")
            nc.tensor.transpose(fvT_p[:], fv[:, 128 * x:128 * x + 128], ident[:])
            fvT = sbq.tile([128, P], F32, tag="fvT")
            nc.scalar.copy(fvT[:], fvT_p[:])
            nv_p = ps.tile([P, 32], F32, tag="ps_small")
            nc.tensor.matmul(nv_p[:], fvT[:], ntWv[:], start=True, stop=True)
            nc.scalar.copy(nv[:, 32 * x:32 * x + 32], nv_p[:])

        # ---- dtp2 ----
        b2r = dtp(h2T['ef'], W3ef, es_et[:, :], ev_et[:, :], full=False)
        as_ = sbq.tile([P, 96], F32, tag="as")
        nc.vector.scalar_tensor_tensor(as_[:, 0:64], b2r['bil_ss'][:], 0.125,
                                       ns[:], op0=OP.mult, op1=OP.mult)
        t96b = sbq.tile([P, 96], F32, tag="t96b")
        nc.vector.scalar_tensor_tensor(
            ap3(t96b, [[3, 32], [1, 3]]),
            ap3(nv, [[1, 32], [32, 3]]), 96.0 ** -0.5,
            ap3(b2r['bv0'], [[3, 32], [1, 3]]), op0=OP.mult, op1=OP.mult)
        nc.vector.tensor_reduce(as_[:, 64:96], ap3(t96b, [[3, 32], [1, 3]]),
                                axis=AX.X, op=OP.add)
        nc.vector.tensor_tensor(as_[:], as_[:], rows_slice(rep, 'ef_bias'), op=OP.add)

        # ---- adaLN ----
        mu = sb.tile([P, 1], F32, tag="amu")
        nc.vector.tensor_reduce(mu[:], as_[:], axis=AX.X, op=OP.add)
        nc.vector.tensor_scalar_mul(mu[:], mu[:], 1.0 / S_TP)
        cen = sbq.tile([P, S_TP], F32, tag="acen")
        nc.vector.tensor_scalar(cen[:], as_[:], mu[:, :1], None, op0=OP.subtract)
        sqv = sbq.tile([P, S_TP], F32, tag="asq")
        var = sb.tile([P, 1], F32, tag="avar")
        nc.scalar.activation(sqv[:], cen[:], ACTF.Square, accum_out=var[:])
        std = sb.tile([P, 1], F32, tag="astd")
        nc.scalar.activation(std[:], var[:], ACTF.Sqrt, scale=1.0 / S_TP,
                             bias=rep[:, ROWS['eps'][0]:ROWS['eps'][0] + 1])
        rstd = sb.tile([P, 1], F32, tag="arstd")
        nc.vector.reciprocal(rstd[:], std[:])
        s_n = sbq.tile([P, S_TP], F32, tag="sn")
        nc.vector.scalar_tensor_tensor(s_n[:], cen[:], rstd[:, :1],
                                       g_mod[:, S_TP:2 * S_TP],
                                       op0=OP.mult, op1=OP.mult)
        nc.vector.tensor_tensor(s_n[:], s_n[:], g_mod[:, 0:S_TP], op=OP.add)

        # ---- scalar head ----
        snT_p = ps.tile([S_TP, P], F32, tag="ps_small")
        nc.tensor.transpose(snT_p[:], s_n[:], ident[:])
        snT = sbq.tile([S_TP, P], F32, tag="snT")
        nc.scalar.copy(snT[:], snT_p[:])
        hd_p = ps.tile([P, 32], F32, tag="ps_small")
        nc.tensor.matmul(hd_p[:], snT[:], spW1[:], start=True, stop=True)
        hd = sb.tile([P, 32], F32, tag="hd")
        evac_add(hd[:], hd_p[:], rows_slice(rep, 'sp_b1'))
        silu(hd[:], hd[:], 32, sb)
        swt = sb.tile([P, 32], F32, tag="swt")
        nc.vector.tensor_tensor(swt[:], hd[:], rows_slice(rep, 'spW2r'), op=OP.mult)
        swr = sb.tile([P, 1], F32, tag="swr")
        nc.vector.tensor_reduce(swr[:], swt[:], axis=AX.X, op=OP.add)
        sw = sb.tile([P, 1], F32, tag="sw")
        nc.vector.tensor_scalar(sw[:], swr[:], 32.0 ** -0.5,
                                rep[:, ROWS['sp_b2'][0]:ROWS['sp_b2'][0] + 1],
                                op0=OP.mult, op1=OP.add)
        den = sb.tile([P, 1], F32, tag="den")
        nc.vector.scalar_tensor_tensor(den[:], d_col[:], 1.0, d_col[:],
                                       op0=OP.add, op1=OP.mult)
        rden = sb.tile([P, 1], F32, tag="rden")
        nc.vector.reciprocal(rden[:], den[:])
        coef = sb.tile([P, 1], F32, tag="coef")
        nc.vector.tensor_mul(coef[:], sw[:], rden[:])
        force = sb.tile([P, 3], F32, tag="force")
        nc.vector.tensor_scalar(force[:], rv[:], coef[:, :1], None, op0=OP.mult)

        if DEBUG:
            dma(T['dbg_force'][e0:e0 + P, :], force[:])
            dma(T['dbg_fs'][e0:e0 + P, :], fs[:])
            dma(T['dbg_as'][e0:e0 + P, :], as_[:])
            dma(T['dbg_gsrc'][e0:e0 + P, :], g_src[:])
            dma(T['dbg_h2'][:, e0:e0 + P], h2T['nf'][:])
            dma(T['dbg_fv'][e0:e0 + P, :], fv[:])
            dma(T['dbg_ns'][e0:e0 + P, :], ns[:])
            dma(T['dbg_nv'][e0:e0 + P, :], nv[:])
            dma(T['dbg_sn'][e0:e0 + P, :], s_n[:])
            dma(T['dbg_eset'][e0:e0 + P, :], es_et[:])
            dma(T['dbg_evet'][e0:e0 + P, :], ev_et[:])

        # ---- scatter: one-hot matmuls into persistent accumulator ----
        acc_p = psa.tile([P, NCHUNK * 3], F32)
        for ch in range(NCHUNK):
            ssh = sb.tile([P, 1], F32, tag="ssh")
            nc.vector.tensor_scalar_add(ssh[:], srcf[:], float(-P * ch))
            oh = sb.tile([P, P], F32, tag="oh")
            nc.vector.tensor_scalar(oh[:], iota_f[:], ssh[:, :1], None,
                                    op0=OP.is_equal)
            nc.tensor.matmul(acc_p[:, 3 * ch:3 * ch + 3], oh[:], force[:],
                             start=True, stop=True, skip_group_check=True)
        nc.vector.tensor_add(acc_sb[:], acc_sb[:], acc_p[:])

    # ---------------- final: evac accumulator ----------------
    for ch in range(NCHUNK):
        dma(T['out'][ch * P:(ch + 1) * P, :], acc_sb[:, 3 * ch:3 * ch + 3])


# ======================= host side =======================

def host_prep(inp):
    inp = {k: np.asarray(v) for k, v in inp.items()}
    src = inp['edge_index'][0].astype(np.int32)
    dst = inp['edge_index'][1].astype(np.int32)
    perm = np.argsort(src, kind='stable')
    src, dst = src[perm], dst[perm]
    gid = inp['batch'].astype(np.int32)[src]
    h_edge = inp['h_edge'][perm]
    dist = inp['distance'][perm].astype(np.float32)
    rvec = inp['relative_vec'][perm].astype(np.float32)

    rows = np.zeros(RWID, np.float32)

    def setr(name, val):
        off, w = ROWS[name]
        rows[off:off + w] = val
    for p in ('nf', 'ef'):
        for q in ('g1', 'b1', 'g2', 'b2'):
            setr(f'{p}_{q}', inp[f'{p}_{q}'])
    setr('src_bs', inp['src_bs']); setr('dst_bs', inp['dst_bs'])
    setr('nt_bs', inp['nt_bs']); setr('et_bs', inp['et_bs'])
    setr('nf_bias', inp['nf_bias']); setr('ef_bias', inp['ef_bias'])
    setr('sp_b1', inp['sp_b1']); setr('spW2r', inp['sp_W2'][:, 0])
    rows[ROWS['sp_b2'][0]] = inp['sp_b2'][0]
    rows[ROWS['eps'][0]] = 1e-5
    setr('normbt', inp['norm_bt'][:2 * S_TP])

    W3ef = inp['ef_W3']
    shared = dict(
        hn_T=np.ascontiguousarray(inp['h_node'].T),
        t_T=np.ascontiguousarray(inp['t'].T),
        normWt=np.ascontiguousarray(inp['norm_Wt'][:, :2 * S_TP]),
        rows=rows.reshape(1, -1),
        rbf_mean_r=inp['rbf_mean'].reshape(1, -1).astype(np.float32),
        rbf_std_r=inp['rbf_std'].reshape(1, -1).astype(np.float32),
        rbf_std_c=inp['rbf_std'].reshape(-1, 1).astype(np.float32),
        rbf_w=inp['rbf_w'].reshape(1, 1).astype(np.float32),
        rbf_b=inp['rbf_b'].reshape(1, 1).astype(np.float32),
        nf_W1=inp['nf_W1'], nf_W2=inp['nf_W2'],
        ef_W1=inp['ef_W1'], ef_W2=inp['ef_W2'],
        W3nf=np.ascontiguousarray(inp['nf_W3']),
        W3ef=np.ascontiguousarray(
            np.concatenate([W3ef[:, :4096], W3ef[:, 8192:9216]], axis=1)),
        src_Ws=inp['src_Ws'], dst_Ws=inp['dst_Ws'],
        src_Wv=inp['src_Wv'], dst_Wv=inp['dst_Wv'],
        nt_Ws=inp['nt_Ws'], nt_Wv=inp['nt_Wv'],
        et_Ws=inp['et_Ws'], et_Wv=inp['et_Wv'],
        sp_W1=inp['sp_W1'],
    )
    shared = {k: np.ascontiguousarray(v, dtype=np.float32) for k, v in shared.items()}

    in_maps = []
    for c in range(NC_CORES):
        sl = slice(c * EC, (c + 1) * EC)
        m = dict(shared)
        m['he_T'] = np.ascontiguousarray(h_edge[sl].T, dtype=np.float32)
        m['dist'] = dist[sl].reshape(-1, 1)
        m['rvec'] = rvec[sl]
        m['srcf'] = src[sl].reshape(-1, 1).astype(np.float32)
        m['srci'] = np.ascontiguousarray(src[sl].reshape(-1, 1))
        m['dsti'] = np.ascontiguousarray(dst[sl].reshape(-1, 1))
        m['gidi'] = np.ascontiguousarray(gid[sl].reshape(-1, 1))
        in_maps.append(m)
    return in_maps


_CACHED_NC = None


def kernel(**inputs):
    global _CACHED_NC
    from concourse.bass_utils import run_bass_kernel_spmd
    if _CACHED_NC is None:
        _CACHED_NC = build_nc()
    in_maps = host_prep(inputs)
    res = run_bass_kernel_spmd(_CACHED_NC, in_maps, list(range(NC_CORES)))
    out = np.zeros((N, 3), np.float32)
    for r in res.results:
        out += r['out']
    return out

